# revision 12
# baseline (speedup 1.0000x reference)
"""Trainium2 Bass kernel for AttentionalFactorizationMachine.

kernel(**inputs) takes FULL unsharded inputs, returns FULL [2048, 1] output.
Internally: data-parallel over 8 NeuronCores (batch sharded, weights
replicated), one SPMD Bass program.

Per-core algorithm (256 items, 780 pairs padded to 784):
  out[b] = (sum_p E_p * g_p) / (sum_p E_p) + fc_b
    E_p = exp(l_p)                       [proj_b dropped: softmax-invariant]
    l_p = proj_w . relu(attn_w^T (x_i*x_j) + attn_b)
    g_p = fc_w . (x_i*x_j)
  Device computes num/den per item; host does the final divide + fc_b.

Layouts (SBUF [partition, free]):
  X_T [ (half,d)=128, (field,b_q)=40*128 ] fp16   via DMA transpose
  ip  [ (half,d)=128, (pair_loc,b_q)     ] fp16   DVE broadcast tensor_mul (2x)
Pipeline per supertile (16 pairs):
  L1:  8 concurrent matmuls (2 b-halves x 4 col-groups, tile_position),
       lhsT=attn_w -> psum_t[h] [(cgrp,a)=128, 512]
  ACT: fused bias+relu psum->SBUF H fp16
  L2': H 128-col chunks as lhsT (transposed trick), rhs=proj4 [128,4]
       -> psum_lg [b_q=128, pair-cols]   (dense logits)
  g:   ip 128-col chunks as lhsT, rhs=fcw2 [128,2]
       -> psum_g [b_q=128, pair-cols]    (dense values)
Per round (16 supertiles): ACT exp -> E + accum_out (den partials);
DVE tensor_tensor_reduce(E*g) -> num partials (chained).
"""

import numpy as np

B, F, D, A = 2048, 40, 64, 32
N_CORES = 8
BC = B // N_CORES          # 256 items per core
BQ = 128                   # items per half
N_HALF = 2
PAIRS = F * (F - 1) // 2   # 780
ST_PAIRS = 16
N_ST = (PAIRS + ST_PAIRS - 1) // ST_PAIRS       # 49
PAIRS_PAD = N_ST * ST_PAIRS                     # 784
ROUND_ST = 16
N_ROUNDS = (N_ST + ROUND_ST - 1) // ROUND_ST    # 4
NEG_BIG = -1.0e30

_ROW, _COL = np.triu_indices(F, k=1)


def _pos(p_local: int) -> int:
    # L2' chunk j with rhs col m' produces pair_local=4m'+j at col 4j+m'
    return 4 * (p_local % 4) + p_local // 4


def _patch_tile_drain():
    """This walrus build accepts only ONE sync wait per instruction; split the
    TileContext exit drain into a chain of single-wait drains."""
    import bass_rust
    import concourse.tile as tile_mod
    from concourse.tile import TileContext

    if getattr(TileContext, "_drain_patched", False):
        return

    def _drain_and_barrier(self, tick_clock, wait_clock):
        drain_inst = self.nc.sync.drain()
        wait_clock.add_sem_waits(
            drain_inst.ins, tile_mod.ScopedClock({None: tick_clock.global_clock})
        )
        si = drain_inst.ins.sync_info
        if si is not None and len(si.on_wait) > 1:
            waits = list(si.on_wait)
            drain_inst.ins.sync_info = bass_rust.SyncInfo(
                on_wait=[waits[0]], on_update=list(si.on_update)
            )
            for w in waits[1:]:
                extra = self.nc.sync.drain()
                extra.ins.sync_info = bass_rust.SyncInfo(on_wait=[w], on_update=[])

    TileContext._drain_and_barrier = _drain_and_barrier
    TileContext._drain_patched = True


def _split_multiwait(nc):
    """Walrus here allows ONE sync wait per instruction: move surplus waits
    onto same-engine NoOps inserted immediately before the instruction."""
    import concourse.mybir as mybir

    for f in nc.m.functions:
        for blk in f.blocks:
            il = blk.instructions
            idx = 0
            while idx < len(il):
                inst = il[idx]
                si = inst.sync_info
                if si is not None and len(si.on_wait) > 1:
                    waits = list(si.on_wait)
                    inst.sync_info = mybir.SyncInfo(
                        on_wait=[waits[-1]], on_update=list(si.on_update)
                    )
                    for k, w in enumerate(waits[:-1]):
                        nop = mybir.InstNoOp(
                            name=f"{inst.name}_w{k}",
                            sync_info=mybir.SyncInfo(on_wait=[w], on_update=[]),
                            bass_nofuse=True,
                            engine=inst.engine,
                        )
                        il.insert(idx, nop)
                        idx += 1
                idx += 1


def build_core_program(split_waits=True):
    """The single-core SPMD Bass program (identical on all 8 cores).

    split_waits: rewrite multi-wait instructions for walrus (HW path).
    CoreSim cannot consume the rewritten program; pass False for sim."""
    import concourse.bass as bass
    import concourse.mybir as mybir
    from concourse.tile import TileContext

    _patch_tile_drain()
    dt = mybir.dt
    AF = mybir.ActivationFunctionType
    ALU = mybir.AluOpType

    nc = bass.Bass()
    x_in = nc.dram_tensor("x", [BC, F, D], dt.float32, kind="ExternalInput")
    attn_w_in = nc.dram_tensor("attn_w", [D, A], dt.float32, kind="ExternalInput")
    attn_b_in = nc.dram_tensor("attn_b", [A], dt.float32, kind="ExternalInput")
    proj_w_in = nc.dram_tensor("proj_w", [A, 1], dt.float32, kind="ExternalInput")
    fc_w_in = nc.dram_tensor("fc_w", [D, 1], dt.float32, kind="ExternalInput")
    out_t = nc.dram_tensor("out", [BC, 2], dt.float32, kind="ExternalOutput")

    # fp16 field-major scratch for DMA transpose: [h, f, b_q, d]
    scratch = nc.dram_tensor("scratch", [N_HALF, F, BQ, D], dt.float16)

    # per-supertile segments: ("tt", i, j0, j1, p_local0) or ("pad", ...)
    seg_of_st = []
    for s in range(N_ST):
        p_lo, p_hi = s * ST_PAIRS, (s + 1) * ST_PAIRS
        segs, p = [], p_lo
        while p < p_hi:
            if p < PAIRS:
                i, j = int(_ROW[p]), int(_COL[p])
                run = min(p_hi, PAIRS, p + (F - j)) - p
                segs.append(("tt", i, j, j + run, p - p_lo))
                p += run
            else:
                segs.append(("pad", 0, 0, p_hi - p, p - p_lo))
                p = p_hi
        seg_of_st.append(segs)

    with TileContext(nc) as tc:
        with (
            tc.tile_pool(name="const", bufs=1) as cpool,
            tc.tile_pool(name="xstage", bufs=1) as xpool,
            tc.tile_pool(name="ip", bufs=3) as ippool,
            tc.tile_pool(name="hbuf", bufs=3) as hpool,
            tc.tile_pool(name="escr", bufs=2) as epool,
            tc.tile_pool(name="acc", bufs=1) as apool,
            tc.tile_pool(name="pst", bufs=2, space="PSUM") as pst,
            tc.tile_pool(name="plg", bufs=2, space="PSUM") as plg,
            tc.tile_pool(name="pgv", bufs=2, space="PSUM") as pgv,
        ):
            # ---------- weights prep ----------
            aw_f32 = cpool.tile([128, A], dt.float32)
            for h in range(N_HALF):
                nc.sync.dma_start(aw_f32[64 * h:64 * (h + 1), :], attn_w_in[:, :])
            aw = cpool.tile([128, A], dt.float16)
            nc.gpsimd.tensor_copy(aw[:], aw_f32[:])

            ab = cpool.tile([128, 1], dt.float32)  # attn_b tiled 4x
            for c in range(4):
                nc.sync.dma_start(
                    ab[32 * c:32 * (c + 1), :],
                    attn_b_in[:].rearrange("(a o) -> a o", o=1),
                )

            p4_f32 = cpool.tile([128, 4], dt.float32)
            nc.vector.memset(p4_f32[:], 0.0)
            for m in range(4):
                nc.sync.dma_start(
                    p4_f32[32 * m:32 * (m + 1), m:m + 1], proj_w_in[:, :]
                )
            proj4 = cpool.tile([128, 4], dt.float16)
            nc.gpsimd.tensor_copy(proj4[:], p4_f32[:])

            f2_f32 = cpool.tile([128, 2], dt.float32)
            nc.vector.memset(f2_f32[:], 0.0)
            for h in range(N_HALF):
                nc.sync.dma_start(
                    f2_f32[64 * h:64 * (h + 1), h:h + 1], fc_w_in[:, :]
                )
            fcw2 = cpool.tile([128, 2], dt.float16)
            nc.gpsimd.tensor_copy(fcw2[:], f2_f32[:])

            # ---------- x prep ----------
            x_f32 = xpool.tile([BQ, N_HALF * F * D], dt.float32)
            nc.sync.dma_start(
                x_f32[:].rearrange("q (h f d) -> q h f d", h=N_HALF, f=F),
                x_in[:].rearrange("(h q) f d -> q h f d", h=N_HALF),
            )
            x_f16 = xpool.tile([BQ, N_HALF * F * D], dt.float16)
            nc.gpsimd.tensor_copy(x_f16[:], x_f32[:])
            nc.sync.dma_start(
                scratch[:].rearrange("h f q d -> q h f d"),
                x_f16[:].rearrange("q (h f d) -> q h f d", h=N_HALF, f=F),
            )
            xt = xpool.tile([128, F * BQ], dt.float16)
            for h in range(N_HALF):
                nc.sync.dma_start_transpose(
                    xt[64 * h:64 * (h + 1), :],
                    scratch[h:h + 1].rearrange("o f q d -> (o f q) d"),
                )

            # ---------- accumulators ----------
            # partials laid out [128, (h, r)]: col h*N_ROUNDS + r
            num_parts = apool.tile([128, 2 * N_ROUNDS], dt.float32)
            den_parts = apool.tile([128, 2 * N_ROUNDS], dt.float32)

            for r in range(N_ROUNDS):
                st0, st1 = r * ROUND_ST, min((r + 1) * ROUND_ST, N_ST)
                n_st_r = st1 - st0
                lg_ps = plg.tile([128, 512], dt.float32, tag="lg")
                g_ps = pgv.tile([128, 512], dt.float32, tag="gv")

                for s in range(st0, st1):
                    sl = s - st0
                    # ---- ip build ----
                    ip = ippool.tile([128, ST_PAIRS * BQ], dt.float16, tag="ip")
                    for kind, i, j0, j1, pl in seg_of_st[s]:
                        if kind == "pad":
                            npd = j1 - j0
                            nc.vector.memset(ip[:, pl * BQ:(pl + npd) * BQ], 0.0)
                            continue
                        njj = j1 - j0
                        nc.vector.tensor_mul(
                            ip[:, pl * BQ:(pl + njj) * BQ].rearrange(
                                "p (j q) -> p j q", j=njj
                            ),
                            xt[:, j0 * BQ:j1 * BQ].rearrange(
                                "p (j q) -> p j q", j=njj
                            ),
                            xt[:, i * BQ:(i + 1) * BQ].rearrange(
                                "p (o q) -> p o q", o=1
                            ).broadcast_to((128, njj, BQ)),
                        )

                    # ---- L1: 8 concurrent matmuls ----
                    ps_t = [
                        pst.tile([128, 512], dt.float32, tag=f"t{h}",
                                 name=f"ps_t{h}_{s}")
                        for h in range(N_HALF)
                    ]
                    for h in range(N_HALF):
                        for c in range(4):
                            nc.tensor.matmul(
                                ps_t[h][32 * c:32 * (c + 1), :],
                                aw[64 * h:64 * (h + 1), :],
                                ip[64 * h:64 * (h + 1), 512 * c:512 * (c + 1)],
                                start=True, stop=True,
                                tile_position=(64 * h, 32 * c),
                            )

                    # ---- relu + bias ----
                    hs = [
                        hpool.tile([128, 512], dt.float16, tag=f"h{h}",
                                   name=f"hs{h}_{s}")
                        for h in range(N_HALF)
                    ]
                    for h in range(N_HALF):
                        nc.scalar.activation(
                            hs[h][:], ps_t[h][:], AF.Relu, bias=ab[:], scale=1.0
                        )

                    # ---- L2' dense logits ----
                    for h in range(N_HALF):
                        for j in range(4):
                            c0 = 32 * sl + 16 * h + 4 * j
                            nc.tensor.matmul(
                                lg_ps[:, c0:c0 + 4],
                                hs[h][:, 128 * j:128 * (j + 1)],
                                proj4[:],
                                start=True, stop=True,
                            )

                    # ---- g dense ----
                    for pl in range(ST_PAIRS):
                        c0 = 32 * sl + 2 * _pos(pl)
                        nc.tensor.matmul(
                            g_ps[:, c0:c0 + 2],
                            ip[:, BQ * pl:BQ * (pl + 1)],
                            fcw2[:],
                            start=True, stop=True,
                        )

                # ---- poison pad logits -> exp gives 0 ----
                if st1 == N_ST and PAIRS_PAD > PAIRS:
                    sl = (N_ST - 1) - st0
                    for h in range(N_HALF):
                        # pads p_local 12..15 -> cols pos {3,7,11,15}
                        base = 32 * sl + 16 * h
                        nc.vector.memset(
                            lg_ps[:, base + 3:base + 16:4], NEG_BIG
                        )

                # ---- exp (den) + E*g reduce (num) ----
                e_sb = epool.tile([128, 512], dt.float32, tag="E")
                tt_scr = epool.tile([128, 256], dt.float32, tag="ttscr")
                for h in range(N_HALF):
                    # iterate (c, s): non-mergeable dims so all APs
                    # canonicalize to the same shape in sim and on DVE
                    l_ap = lg_ps[:].rearrange(
                        "q (s hh c) -> q c hh s", hh=2, c=16
                    )[:, :, h, 0:n_st_r]
                    e_ap = e_sb[:].rearrange(
                        "q (s hh c) -> q c hh s", hh=2, c=16
                    )[:, :, h, 0:n_st_r]
                    g_ap = g_ps[:].rearrange(
                        "q (s c hh) -> q c hh s", c=16, hh=2
                    )[:, :, h, 0:n_st_r]
                    col = h * N_ROUNDS + r
                    nc.scalar.activation(
                        e_ap, l_ap, AF.Exp,
                        accum_out=den_parts[:, col:col + 1],
                    )
                    nc.vector.scalar_tensor_tensor(
                        out=tt_scr[:, 0:16 * n_st_r].rearrange(
                            "q (c s) -> q c s", s=n_st_r
                        ),
                        in0=e_ap,
                        scalar=1.0,
                        in1=g_ap,
                        op0=ALU.mult,
                        op1=ALU.mult,
                        accum_out=num_parts[:, col:col + 1],
                    )

            # ---------- epilogue: reduce partials, emit num/den ----------
            nd = apool.tile([128, 4], dt.float32)  # cols (h, k): num,den
            nc.vector.tensor_reduce(
                nd[:, 0:4:2],
                num_parts[:].rearrange("q (h r) -> q h r", h=2),
                mybir.AxisListType.X, ALU.add,
            )
            nc.vector.tensor_reduce(
                nd[:, 1:4:2],
                den_parts[:].rearrange("q (h r) -> q h r", h=2),
                mybir.AxisListType.X, ALU.add,
            )
            nc.sync.dma_start(
                out_t[:].rearrange("(h q) k -> q h k", h=2),
                nd[:].rearrange("q (h k) -> q h k", h=2),
            )

    if split_waits:
        _split_multiwait(nc)
    return nc


_CACHED = {}


def _get_runner():
    if "runner" in _CACHED:
        return _CACHED["runner"]
    import jax
    from jax.sharding import Mesh, PartitionSpec
    from jax.experimental.shard_map import shard_map
    import concourse.mybir as mybir
    from concourse.bass2jax import (
        _bass_exec_p, install_neuronx_cc_hook, partition_id_tensor,
    )

    nc = build_core_program()
    install_neuronx_cc_hook()

    partition_name = nc.partition_id_tensor.name if nc.partition_id_tensor else None
    in_names, out_names, out_avals, zero_outs = [], [], [], []
    for alloc in nc.m.functions[0].allocations:
        if not isinstance(alloc, mybir.MemoryLocationSet):
            continue
        name = alloc.memorylocations[0].name
        if alloc.kind == "ExternalInput":
            if name != partition_name:
                in_names.append(name)
        elif alloc.kind == "ExternalOutput":
            out_names.append(name)
            shape = tuple(alloc.tensor_shape)
            dtype = mybir.dt.np(alloc.dtype)
            out_avals.append(jax.core.ShapedArray(shape, dtype))
            zero_outs.append(np.zeros(shape, dtype))
    n_params = len(in_names)
    n_outs = len(out_avals)
    all_in = in_names + out_names + ([partition_name] if partition_name else [])

    def _body(*args):
        operands = list(args)
        if partition_name is not None:
            operands.append(partition_id_tensor())
        outs = _bass_exec_p.bind(
            *operands,
            out_avals=tuple(out_avals),
            in_names=tuple(all_in),
            out_names=tuple(out_names),
            lowering_input_output_aliases=(),
            sim_require_finite=True,
            sim_require_nnan=True,
            nc=nc,
        )
        return tuple(outs)

    devices = jax.devices()[:N_CORES]
    mesh = Mesh(np.asarray(devices), ("core",))
    fn = jax.jit(
        shard_map(
            _body, mesh=mesh,
            in_specs=(PartitionSpec("core"),) * (n_params + n_outs),
            out_specs=(PartitionSpec("core"),) * n_outs,
            check_rep=False,
        ),
        keep_unused=True,
    )
    _CACHED["runner"] = {
        "fn": fn, "in_names": in_names, "out_names": out_names,
        "zero_outs": zero_outs, "mesh": mesh, "nc": nc,
    }
    return _CACHED["runner"]


def _device_args(r, x, attn_w, attn_b, proj_w, fc_w):
    feeds = {
        "x": np.ascontiguousarray(
            np.asarray(x, np.float32).reshape(N_CORES * BC, F, D)
        ),
        "attn_w": np.ascontiguousarray(
            np.tile(np.asarray(attn_w, np.float32), (N_CORES, 1))
        ),
        "attn_b": np.ascontiguousarray(
            np.tile(np.asarray(attn_b, np.float32), N_CORES)
        ),
        "proj_w": np.ascontiguousarray(
            np.tile(np.asarray(proj_w, np.float32), (N_CORES, 1))
        ),
        "fc_w": np.ascontiguousarray(
            np.tile(np.asarray(fc_w, np.float32), (N_CORES, 1))
        ),
    }
    concat_in = [feeds[n] for n in r["in_names"]]
    concat_zeros = [
        np.zeros((N_CORES * z.shape[0], *z.shape[1:]), z.dtype)
        for z in r["zero_outs"]
    ]
    return concat_in + concat_zeros


def kernel(x, attn_w, attn_b, proj_w, proj_b, fc_w, fc_b):
    """FULL inputs -> FULL output. proj_b is softmax-invariant (unused)."""
    import jax

    r = _get_runner()
    args = _device_args(r, x, attn_w, attn_b, proj_w, fc_w)
    outs = r["fn"](*args)
    jax.block_until_ready(outs)
    nd = np.asarray(outs[r["out_names"].index("out")]).reshape(B, 2)
    fc_b = np.asarray(fc_b, np.float32)
    return (nd[:, 0] / nd[:, 1] + fc_b[0]).astype(np.float32)[:, None]
